# revision 15
# baseline (speedup 1.0000x reference)
"""
Trainium2 Bass kernel for nn_DisjointDecoderAE.

  encoder (shared MLP):  x[B,U] -> relu x3 -> z[B,L]
  decoder (U disjoint MLPs, stacked weights): z -> relu(64) -> relu(64) -> relu(64) -> scalar

Sharding: encoder replicated on every core (it is tiny); decoder expert-parallel
over the unit axis U (64 units per core x 8 cores).

The kernel is drain-bound: every decoder activation element must pass through
a PSUM->SBUF relu+bias op, and only VectorE (0.96 G elem/s) and ScalarE
(1.2 G elem/s) can read PSUM on TRN2 (GpSimd and DMA cannot).  216 drains of
[128,1024] => ~128.6us floor.  The PE port floor (128 in / 128 out lanes per
cycle) is ~118us, so both must stay saturated:

  * matmuls are [64,64]-quadrant packed (tile_position) so LDWEIGHTS of one
    quadrant overlaps streaming of another -- full-width matmuls serialize
    the weight load (~+160ns each, measured);
  * the decoder is a flat software pipeline over 32 subgroup-slots
    (L1(t) | L2(t-1) | L3(t-2) | L4(t-3)), so drains never wait and the
    head/tail bubbles of the per-group schedule are gone;
  * drains alternate VectorE/ScalarE weighted by measured per-op cost.

Self-contained: shapes/sharding hardcoded; host packs weights, device
computes, host re-assembles (final transpose + bd4 bias on host).
"""

import os
import sys

sys.path.insert(0, "/opt/trn_rl_repo")

import numpy as np
import ml_dtypes

import concourse.bass as bass
import concourse.mybir as mybir
import concourse.tile as tile
from concourse import bacc
from concourse.bass_utils import run_bass_kernel_spmd

B, U, L, H = 2048, 512, 32, 64
NCORES = 8
UC = U // NCORES          # 64 units per core
NG = UC // 8              # 8 groups of 8 units (L1/L4 packing)
NS = UC // 4              # 16 subgroups of 4 units (pipeline slots)
CH = 512                  # one fp32 PSUM bank of batch
CP = 1024                 # chunk-pair (drain granularity, 2 banks)
NCP = B // CP             # 2 chunk-pairs
KT = U // 128             # 4 k-tiles for encoder layer 1
NSLOT = NCP * NS          # 32 pipeline slots

BF16 = mybir.dt.bfloat16
FP32 = mybir.dt.float32
NPBF = ml_dtypes.bfloat16

# L2/L3 per-subgroup placement tables (j = unit index within subgroup of 4).
# ih = input partition half, oh = output partition half, tsel = which input
# tile of the subgroup's pair, bank = which output psum tensor (0=A, 1=B).
IH2 = (0, 1, 0, 1)
OH2 = (0, 1, 1, 0)
TS2 = (0, 0, 1, 1)
BK2 = (0, 0, 1, 1)
IH3 = (0, 1, 1, 0)
OH3 = (0, 1, 0, 1)
TS3 = (0, 0, 1, 1)
BK3 = (0, 0, 1, 1)

LAST_EXEC_NS = None
LAST_RESULTS = None
_PROG = None


def _pack_shared(x, We1, be1, We2, be2, We3, be3, We4, be4):
    xT = np.ascontiguousarray(x.T).astype(NPBF)              # [U, B]
    xt = np.ascontiguousarray(xT.reshape(KT, 128, B))        # k-tiles
    wenc = np.zeros((128, 512), np.float32)
    wenc[:, 0:KT * H] = We1.reshape(KT, 128, H).transpose(1, 0, 2).reshape(
        128, KT * H)
    wenc[0:H, 256:320] = We2
    wenc[0:H, 320:384] = We3
    wenc[0:H, 384:512] = np.tile(We4, (1, 4))
    benc = np.zeros((128, 4), np.float32)
    benc[0:H, 0] = be1
    benc[0:H, 1] = be2
    benc[0:H, 2] = be3
    benc[:, 3] = np.tile(be4, 4)
    return dict(xt=xt, wenc=wenc.astype(NPBF), benc=benc)


def _pack_core(c, Wd1, bd1, Wd2, bd2, Wd3, bd3, Wd4):
    u0 = c * UC
    w1 = Wd1[u0:u0 + UC]
    b1 = bd1[u0:u0 + UC]
    w2 = Wd2[u0:u0 + UC]
    b2 = bd2[u0:u0 + UC]
    w3 = Wd3[u0:u0 + UC]
    b3 = bd3[u0:u0 + UC]
    w4 = Wd4[u0:u0 + UC]

    # L1: one [32,128] lhsT per unit-pair q (both units share rhs z):
    # row strip q%4, col block q//4 = group index.
    wd1p = np.zeros((128, NG * 2 * H), np.float32)
    bd1p = np.zeros((128, UC // 2), np.float32)
    for q in range(UC // 2):
        r = q % 4
        blk = q // 4
        wd1p[32 * r:32 * r + 32, blk * 128:blk * 128 + 64] = w1[2 * q]
        wd1p[32 * r:32 * r + 32, blk * 128 + 64:blk * 128 + 128] = w1[2 * q + 1]
        bd1p[0:64, q] = b1[2 * q]
        bd1p[64:128, q] = b1[2 * q + 1]

    wd2p = np.zeros((128, NS * 2 * H), np.float32)
    wd3p = np.zeros((128, NS * 2 * H), np.float32)
    bd2p = np.zeros((128, NS * 2), np.float32)
    bd3p = np.zeros((128, NS * 2), np.float32)
    for s in range(NS):
        for j in range(4):
            u = 4 * s + j
            blk = 2 * s + (j >> 1)
            wd2p[64 * IH2[j]:64 * IH2[j] + 64, blk * H:(blk + 1) * H] = w2[u]
            wd3p[64 * IH3[j]:64 * IH3[j] + 64, blk * H:(blk + 1) * H] = w3[u]
        # T2 banks: A = {4s lo, 4s+1 hi}; B = {4s+3 lo, 4s+2 hi}
        bd2p[0:64, 2 * s] = b2[4 * s]
        bd2p[64:128, 2 * s] = b2[4 * s + 1]
        bd2p[0:64, 2 * s + 1] = b2[4 * s + 3]
        bd2p[64:128, 2 * s + 1] = b2[4 * s + 2]
        # T3 banks natural
        bd3p[0:64, 2 * s] = b3[4 * s]
        bd3p[64:128, 2 * s] = b3[4 * s + 1]
        bd3p[0:64, 2 * s + 1] = b3[4 * s + 2]
        bd3p[64:128, 2 * s + 1] = b3[4 * s + 3]

    # M padded to 32 (zero cols) so L4 matmuls cover all psum partitions
    wd4p = np.zeros((128, UC // 2 * 32), np.float32)
    for p in range(UC // 2):
        wd4p[0:64, 32 * p] = w4[2 * p]
        wd4p[64:128, 32 * p + 1] = w4[2 * p + 1]

    bdec = np.concatenate([bd1p, bd2p, bd3p], axis=1)        # [128, 96]
    return dict(wd1=wd1p.astype(NPBF), wd2=wd2p.astype(NPBF),
                wd3=wd3p.astype(NPBF), wd4=wd4p.astype(NPBF), bdec=bdec)


class _Drain:
    """Weighted VectorE/ScalarE alternation for PSUM->SBUF drains,
    using HW-measured per-op costs."""

    def __init__(self, nc):
        self.nc = nc
        self.t_dve = 0.0
        self.t_act = 0.0

    def __call__(self, out, psum, bias=None, relu=False):
        fd = 1
        for step, cnt in psum.ap[1:]:
            fd *= cnt
        dve_ns = (120.0 + fd) / 0.96 + 88.0
        act_ns = (172.0 + fd) / 1.2 + 117.0
        nc = self.nc
        if self.t_dve + dve_ns <= self.t_act + act_ns:
            self.t_dve += dve_ns
            if relu:
                nc.vector.tensor_scalar(out, psum, bias, 0.0,
                                        op0=mybir.AluOpType.add,
                                        op1=mybir.AluOpType.max)
            elif bias is not None:
                nc.vector.tensor_scalar(out, psum, bias, None,
                                        op0=mybir.AluOpType.add)
            else:
                nc.vector.tensor_copy(out, psum)
        else:
            self.t_act += act_ns
            if relu:
                nc.scalar.activation(out, psum, mybir.ActivationFunctionType.Relu,
                                     bias=bias)
            elif bias is not None:
                nc.scalar.activation(out, psum,
                                     mybir.ActivationFunctionType.Identity,
                                     bias=bias)
            else:
                nc.scalar.copy(out, psum)


def _build_program():
    nc = bacc.Bacc("TRN2", target_bir_lowering=False, debug=False)

    def din(name, shape, dtype):
        return nc.dram_tensor(name, list(shape), dtype, kind="ExternalInput").ap()

    xt_d = din("xt", (KT, 128, B), BF16)
    wenc_d = din("wenc", (128, 512), BF16)
    benc_d = din("benc", (128, 4), FP32)
    wd1_d = din("wd1", (128, NG * 2 * H), BF16)
    wd2_d = din("wd2", (128, NS * 2 * H), BF16)
    wd3_d = din("wd3", (128, NS * 2 * H), BF16)
    wd4_d = din("wd4", (128, UC // 2 * 32), BF16)
    bdec_d = din("bdec", (128, 96), FP32)
    out_d = nc.dram_tensor("out", [UC, B], FP32, kind="ExternalOutput").ap()

    RELU = True

    with tile.TileContext(nc) as tc:
        with (
            tc.tile_pool(name="const", bufs=1) as const,
            tc.tile_pool(name="h1p", bufs=5) as h1p,
            tc.tile_pool(name="h2p", bufs=5) as h2p,
            tc.tile_pool(name="h3p", bufs=5) as h3p,
            tc.tile_pool(name="stg", bufs=2) as stgp,
            tc.tile_pool(name="ps", bufs=3, space="PSUM") as psp,
            tc.tile_pool(name="pl4", bufs=1, space="PSUM") as pl4p,
        ):
            drain = _Drain(nc)

            def load(dst_shape, dtype, src, tag):
                t = const.tile(list(dst_shape), dtype, tag=tag, name=tag)
                nc.sync.dma_start(out=t[:], in_=src)
                return t

            # PE warm-up burst on memset data: ramps the PE p-state while
            # the input DMAs land.  No DMA dependencies.
            wu = const.tile([128, 512], BF16, tag="wu", name="wu")
            nc.vector.memset(wu[:], 0.0)
            wu_ps = psp.tile([128, CP], FP32, tag="ps", name="wu_ps")
            for i in range(8):
                nc.tensor.matmul(wu_ps[:, (i % 2) * CH:(i % 2) * CH + CH],
                                 wu[:, 0:128], wu[:, 0:CH])

            # encoder weights + x chunk 0 first so the encoder starts early
            wenc = load((128, 512), BF16, wenc_d[:], "wenc")
            benc = load((128, 4), FP32, benc_d[:], "benc")
            we1 = wenc[:, 0:KT * H]
            we2 = wenc[0:H, 256:320]
            we3 = wenc[0:H, 320:384]
            we4 = wenc[0:H, 384:512]
            be1 = benc[0:H, 0:1]
            be2 = benc[0:H, 1:2]
            be3 = benc[0:H, 2:3]
            be4 = benc[:, 3:4]

            xts = [const.tile([128, B], BF16, tag=f"xt{t}", name=f"xt{t}")
                   for t in range(KT)]
            # dma_start issue costs ~0.6us of the issuing engine queue, so
            # spread issues across the (idle at preamble) engine queues and
            # load the encoder-critical slices first.
            qs = [nc.scalar, nc.sync, nc.scalar, nc.sync]
            for t in range(KT):
                qs[t].dma_start(out=xts[t][:, 0:CH], in_=xt_d[t][:, 0:CH])
            for t in range(KT):
                qs[t].dma_start(out=xts[t][:, CH:CP], in_=xt_d[t][:, CH:CP])
            wd1 = const.tile([128, NG * 2 * H], BF16, tag="wd1", name="wd1")
            nc.scalar.dma_start(out=wd1[:], in_=wd1_d[:])
            bdec = const.tile([128, 96], FP32, tag="bdec", name="bdec")
            nc.sync.dma_start(out=bdec[:], in_=bdec_d[:])
            for t in range(KT):
                qs[t].dma_start(out=xts[t][:, CP:B], in_=xt_d[t][:, CP:B])
            wd2 = const.tile([128, NS * 2 * H], BF16, tag="wd2", name="wd2")
            nc.sync.dma_start(out=wd2[:], in_=wd2_d[:])
            wd3 = const.tile([128, NS * 2 * H], BF16, tag="wd3", name="wd3")
            nc.scalar.dma_start(out=wd3[:], in_=wd3_d[:])
            wd4 = const.tile([128, UC // 2 * 32], BF16, tag="wd4", name="wd4")
            nc.sync.dma_start(out=wd4[:], in_=wd4_d[:])
            bd1 = bdec[:, 0:32]
            bd2 = bdec[:, 32:64]
            bd3 = bdec[:, 64:96]

            z1 = const.tile([H, B], BF16, tag="z1", name="z1")
            z2 = const.tile([H, B], BF16, tag="z2", name="z2")
            z3 = const.tile([H, B], BF16, tag="z3", name="z3")
            zr = const.tile([128, B], BF16, tag="zr", name="zr")

            # ---------------- encoder (replicated), 512-chunk stages -----
            # Four parallel chains (one per 512-chunk) with short drains keep
            # the encoder critical path ~6us instead of ~14us.
            def enc_l1c(c):
                c0 = c * CH
                ps = psp.tile([128, CP], FP32, tag="ps", name=f"pe1_{c}")
                for t in range(KT):
                    nc.tensor.matmul(ps[0:H, 0:CH], we1[:, t * H:(t + 1) * H],
                                     xts[t][:, c0:c0 + CH],
                                     start=(t == 0), stop=(t == KT - 1))
                drain(z1[:, c0:c0 + CH], ps[0:H, 0:CH], be1, RELU)

            def enc_midc(c, win, bin_, zin, zout):
                c0 = c * CH
                ps = psp.tile([128, CP], FP32, tag="ps", name=f"pem_{c}")
                nc.tensor.matmul(ps[0:H, 0:CH], win, zin[:, c0:c0 + CH])
                drain(zout[:, c0:c0 + CH], ps[0:H, 0:CH], bin_, RELU)

            def enc_l4c(c):
                c0 = c * CH
                ps = psp.tile([128, CP], FP32, tag="ps", name=f"pe4_{c}")
                nc.tensor.matmul(ps[:, 0:CH], we4, z3[:, c0:c0 + CH])
                drain(zr[:, c0:c0 + CH], ps[:, 0:CH], be4, False)

            NCHUNK = B // CH
            for st in range(NCHUNK + 3):
                for lyr in range(4):
                    c = st - lyr
                    if 0 <= c < NCHUNK:
                        if lyr == 0:
                            enc_l1c(c)
                        elif lyr == 1:
                            enc_midc(c, we2, be2, z1, z2)
                        elif lyr == 2:
                            enc_midc(c, we3, be3, z2, z3)
                        else:
                            enc_l4c(c)

            # ---------------- decoder: flat subgroup pipeline ------------
            # slot = cp*NS + s over subgroups s of 4 units (pairs 2s, 2s+1);
            # stages L1(t) | L2(t-1) | L3(t-2) | L4(t-3).
            T1 = [None] * NSLOT        # [tileA, tileB] per slot
            T2 = [None] * NSLOT
            T3 = [None] * NSLOT
            pl4_tiles = {}

            def l1_stage(sl_):
                cp, s = divmod(sl_, NS)
                tiles = []
                pss = [psp.tile([128, CP], FP32, tag="ps",
                                name=f"pl1_{sl_}_{k}") for k in range(2)]
                for cc in range(2):
                    c0 = cp * CP + cc * CH
                    for k in range(2):
                        q = 2 * s + k
                        r = q % 4
                        blk = q // 4
                        nc.tensor.matmul(
                            pss[k][:, cc * CH:(cc + 1) * CH],
                            wd1[32 * r:32 * r + 32, blk * 128:(blk + 1) * 128],
                            zr[32 * r:32 * r + 32, c0:c0 + CH],
                            tile_position=(32 * r, 0))
                for k in range(2):
                    q = 2 * s + k
                    t_sb = h1p.tile([128, CP], BF16, tag="t1",
                                    name=f"t1_{sl_}_{k}")
                    drain(t_sb[:, :], pss[k][:, :], bd1[:, q:q + 1], RELU)
                    tiles.append(t_sb)
                T1[sl_] = tiles

            def l2_stage(sl_):
                cp, s = divmod(sl_, NS)
                pa = psp.tile([128, CP], FP32, tag="ps", name=f"pa_{sl_}")
                pb = psp.tile([128, CP], FP32, tag="ps", name=f"pb_{sl_}")
                pp = (pa, pb)
                for cc in range(2):
                    c0 = cc * CH
                    for j in range(4):
                        blk = 2 * s + (j >> 1)
                        nc.tensor.matmul(
                            pp[BK2[j]][64 * OH2[j]:64 * OH2[j] + 64,
                                       c0:c0 + CH],
                            wd2[64 * IH2[j]:64 * IH2[j] + 64,
                                blk * H:(blk + 1) * H],
                            T1[sl_][TS2[j]][64 * IH2[j]:64 * IH2[j] + 64,
                                            c0:c0 + CH],
                            tile_position=(64 * IH2[j], 64 * OH2[j]))
                tiles = []
                for k, ps in enumerate(pp):
                    t_sb = h2p.tile([128, CP], BF16, tag="t2",
                                    name=f"t2_{sl_}_{k}")
                    drain(t_sb[:, :], ps[:, :],
                          bd2[:, 2 * s + k:2 * s + k + 1], RELU)
                    tiles.append(t_sb)
                T2[sl_] = tiles
                T1[sl_] = None

            def l3_stage(sl_):
                cp, s = divmod(sl_, NS)
                pa = psp.tile([128, CP], FP32, tag="ps", name=f"pa3_{sl_}")
                pb = psp.tile([128, CP], FP32, tag="ps", name=f"pb3_{sl_}")
                pp = (pa, pb)
                for cc in range(2):
                    c0 = cc * CH
                    for j in range(4):
                        blk = 2 * s + (j >> 1)
                        nc.tensor.matmul(
                            pp[BK3[j]][64 * OH3[j]:64 * OH3[j] + 64,
                                       c0:c0 + CH],
                            wd3[64 * IH3[j]:64 * IH3[j] + 64,
                                blk * H:(blk + 1) * H],
                            T2[sl_][TS3[j]][64 * IH3[j]:64 * IH3[j] + 64,
                                            c0:c0 + CH],
                            tile_position=(64 * IH3[j], 64 * OH3[j]))
                tiles = []
                for k, ps in enumerate(pp):
                    t_sb = h3p.tile([128, CP], BF16, tag="t3",
                                    name=f"t3_{sl_}_{k}")
                    drain(t_sb[:, :], ps[:, :],
                          bd3[:, 2 * s + k:2 * s + k + 1], RELU)
                    tiles.append(t_sb)
                T3[sl_] = tiles
                T2[sl_] = None

            def l4_stage(sl_):
                cp, s = divmod(sl_, NS)
                g = s // 2
                key = (cp, g)
                if s % 2 == 0:
                    pl4_tiles[key] = pl4p.tile([128, CP], FP32, tag="pl4",
                                               name=f"pl4_{cp}_{g}")
                pt = pl4_tiles[key]
                for cc in range(2):
                    for k in range(2):
                        q = 2 * s + k
                        cs = 2 * (s % 2) + k
                        nc.tensor.matmul(
                            pt[32 * cs:32 * cs + 32, cc * CH:(cc + 1) * CH],
                            wd4[:, 32 * q:32 * q + 32],
                            T3[sl_][k][:, cc * CH:(cc + 1) * CH],
                            tile_position=(0, 32 * cs))
                T3[sl_] = None
                if s % 2 == 1:
                    sl_b = slice(cp * CP, (cp + 1) * CP)
                    stg = stgp.tile([128, CP], FP32, tag="stg",
                                    name=f"stg_{cp}_{g}")
                    drain(stg[:, :], pt[:, :], None, False)
                    for cs in range(4):
                        nc.sync.dma_start(
                            out=out_d[8 * g + 2 * cs:8 * g + 2 * cs + 2, sl_b],
                            in_=stg[32 * cs:32 * cs + 2, :])
                    del pl4_tiles[key]

            S = 1
            for t in range(NSLOT + 3 * S):
                if t < NSLOT:
                    l1_stage(t)
                if 0 <= t - S < NSLOT:
                    l2_stage(t - S)
                if 0 <= t - 2 * S < NSLOT:
                    l3_stage(t - 2 * S)
                if 0 <= t - 3 * S < NSLOT:
                    l4_stage(t - 3 * S)

    nc.compile()
    return nc


def _get_program():
    global _PROG
    if _PROG is None:
        _PROG = _build_program()
    return _PROG


def kernel(x, We1, be1, We2, be2, We3, be3, We4, be4,
           Wd1, bd1, Wd2, bd2, Wd3, bd3, Wd4, bd4):
    global LAST_EXEC_NS, LAST_RESULTS
    shared = _pack_shared(np.asarray(x, np.float32),
                          np.asarray(We1, np.float32), np.asarray(be1, np.float32),
                          np.asarray(We2, np.float32), np.asarray(be2, np.float32),
                          np.asarray(We3, np.float32), np.asarray(be3, np.float32),
                          np.asarray(We4, np.float32), np.asarray(be4, np.float32))
    in_maps = []
    for c in range(NCORES):
        m = dict(shared)
        m.update(_pack_core(c, np.asarray(Wd1, np.float32), np.asarray(bd1, np.float32),
                            np.asarray(Wd2, np.float32), np.asarray(bd2, np.float32),
                            np.asarray(Wd3, np.float32), np.asarray(bd3, np.float32),
                            np.asarray(Wd4, np.float32)))
        in_maps.append(m)

    nc = _get_program()
    trace = bool(int(os.environ.get("BASSK_TRACE", "0")))
    kwargs = {}
    if trace:
        kwargs["tmpdir"] = os.environ.get("BASSK_TMPDIR") or None
    res = run_bass_kernel_spmd(nc, in_maps, core_ids=list(range(NCORES)),
                               trace=trace, **kwargs)
    LAST_EXEC_NS = res.exec_time_ns
    LAST_RESULTS = res

    outT = np.concatenate([res.results[c]["out"] for c in range(NCORES)], axis=0)
    out = outT.T.astype(np.float32) + np.asarray(bd4, np.float32)[None, :]
    return out


# revision 16
# speedup vs baseline: 1.1242x; 1.1242x over previous
"""
Trainium2 Bass kernel for nn_DisjointDecoderAE.

  encoder (shared MLP):  x[B,U] -> relu x3 -> z[B,L]
  decoder (U disjoint MLPs, stacked weights): z -> relu(64) -> relu(64) -> relu(64) -> scalar

Sharding: encoder replicated on every core (it is tiny); decoder expert-parallel
over the unit axis U (64 units per core x 8 cores).  All activations live
feature-on-partition / batch-on-free:  h^T [64, B].  Small per-unit matmuls are
packed into the 128x128 PE via tile_position quadrants (concurrent MMs).
PSUM->SBUF drain (relu+bias) is the bottleneck; drains are [128,1024] two-bank
ops (chunk-pair processing) split across VectorE and ScalarE by measured cost.

Self-contained: shapes/sharding hardcoded; host packs weights, device computes,
host re-assembles (final transpose + bd4 bias on host).
"""

import os
import sys

sys.path.insert(0, "/opt/trn_rl_repo")

import numpy as np
import ml_dtypes

import concourse.bass as bass
import concourse.mybir as mybir
import concourse.tile as tile
from concourse import bacc
from concourse.bass_utils import run_bass_kernel_spmd

B, U, L, H = 2048, 512, 32, 64
NCORES = 8
UC = U // NCORES          # 64 units per core
NG = UC // 8              # 8 groups of 8 units
CH = 512                  # one fp32 PSUM bank of batch
CP = 1024                 # chunk-pair (drain granularity, 2 banks)
NCP = B // CP             # 2 chunk-pairs
KT = U // 128             # 4 k-tiles for encoder layer 1

BF16 = mybir.dt.bfloat16
FP32 = mybir.dt.float32
NPBF = ml_dtypes.bfloat16

# L2/L3 per-subgroup placement tables (j = unit index within subgroup of 4).
# ih = input partition half, oh = output partition half, tsel = which input
# tile of the subgroup's pair, bank = which output psum tensor (0=A, 1=B).
IH2 = (0, 1, 0, 1)
OH2 = (0, 1, 1, 0)
TS2 = (0, 0, 1, 1)
BK2 = (0, 0, 1, 1)
IH3 = (0, 1, 1, 0)
OH3 = (0, 1, 0, 1)
TS3 = (0, 0, 1, 1)
BK3 = (0, 0, 1, 1)

LAST_EXEC_NS = None
LAST_RESULTS = None
_PROG = None


def _pack_shared(x, We1, be1, We2, be2, We3, be3, We4, be4):
    xT = np.ascontiguousarray(x.T).astype(NPBF)              # [U, B]
    xt = np.ascontiguousarray(xT.reshape(KT, 128, B))        # k-tiles
    wenc = np.zeros((128, 512), np.float32)
    wenc[:, 0:KT * H] = We1.reshape(KT, 128, H).transpose(1, 0, 2).reshape(
        128, KT * H)
    wenc[0:H, 256:320] = We2
    wenc[0:H, 320:384] = We3
    wenc[0:H, 384:512] = np.tile(We4, (1, 4))
    benc = np.zeros((128, 4), np.float32)
    benc[0:H, 0] = be1
    benc[0:H, 1] = be2
    benc[0:H, 2] = be3
    benc[:, 3] = np.tile(be4, 4)
    return dict(xt=xt, wenc=wenc.astype(NPBF), benc=benc)


def _pack_core(c, Wd1, bd1, Wd2, bd2, Wd3, bd3, Wd4):
    u0 = c * UC
    w1 = Wd1[u0:u0 + UC]
    b1 = bd1[u0:u0 + UC]
    w2 = Wd2[u0:u0 + UC]
    b2 = bd2[u0:u0 + UC]
    w3 = Wd3[u0:u0 + UC]
    b3 = bd3[u0:u0 + UC]
    w4 = Wd4[u0:u0 + UC]

    # L1: one [32,128] lhsT per unit-pair q (both units share rhs z):
    # row strip q%4, col block q//4 = group index.
    wd1p = np.zeros((128, NG * 2 * H), np.float32)
    bd1p = np.zeros((128, UC // 2), np.float32)
    for q in range(UC // 2):
        r = q % 4
        blk = q // 4
        wd1p[32 * r:32 * r + 32, blk * 128:blk * 128 + 64] = w1[2 * q]
        wd1p[32 * r:32 * r + 32, blk * 128 + 64:blk * 128 + 128] = w1[2 * q + 1]
        bd1p[0:64, q] = b1[2 * q]
        bd1p[64:128, q] = b1[2 * q + 1]

    NS = UC // 4  # 16 subgroups
    wd2p = np.zeros((128, NS * 2 * H), np.float32)
    wd3p = np.zeros((128, NS * 2 * H), np.float32)
    bd2p = np.zeros((128, NS * 2), np.float32)
    bd3p = np.zeros((128, NS * 2), np.float32)
    for s in range(NS):
        for j in range(4):
            u = 4 * s + j
            blk = 2 * s + (j >> 1)
            wd2p[64 * IH2[j]:64 * IH2[j] + 64, blk * H:(blk + 1) * H] = w2[u]
            wd3p[64 * IH3[j]:64 * IH3[j] + 64, blk * H:(blk + 1) * H] = w3[u]
        # T2 banks: A = {4s lo, 4s+1 hi}; B = {4s+3 lo, 4s+2 hi}
        bd2p[0:64, 2 * s] = b2[4 * s]
        bd2p[64:128, 2 * s] = b2[4 * s + 1]
        bd2p[0:64, 2 * s + 1] = b2[4 * s + 3]
        bd2p[64:128, 2 * s + 1] = b2[4 * s + 2]
        # T3 banks natural
        bd3p[0:64, 2 * s] = b3[4 * s]
        bd3p[64:128, 2 * s] = b3[4 * s + 1]
        bd3p[0:64, 2 * s + 1] = b3[4 * s + 2]
        bd3p[64:128, 2 * s + 1] = b3[4 * s + 3]

    # M padded to 32 (zero cols) so L4 matmuls cover all psum partitions
    wd4p = np.zeros((128, UC // 2 * 32), np.float32)
    for p in range(UC // 2):
        wd4p[0:64, 32 * p] = w4[2 * p]
        wd4p[64:128, 32 * p + 1] = w4[2 * p + 1]

    bdec = np.concatenate([bd1p, bd2p, bd3p], axis=1)        # [128, 96]
    return dict(wd1=wd1p.astype(NPBF), wd2=wd2p.astype(NPBF),
                wd3=wd3p.astype(NPBF), wd4=wd4p.astype(NPBF), bdec=bdec)


class _Drain:
    """Weighted VectorE/ScalarE alternation for PSUM->SBUF drains,
    using HW-measured per-op costs."""

    def __init__(self, nc):
        self.nc = nc
        self.t_dve = 0.0
        self.t_act = 0.0

    def __call__(self, out, psum, bias=None, relu=False):
        fd = 1
        for step, cnt in psum.ap[1:]:
            fd *= cnt
        dve_ns = (120.0 + fd) / 0.96 + 88.0
        act_ns = (172.0 + fd) / 1.2 + 117.0
        nc = self.nc
        if self.t_dve + dve_ns <= self.t_act + act_ns:
            self.t_dve += dve_ns
            if relu:
                nc.vector.tensor_scalar(out, psum, bias, 0.0,
                                        op0=mybir.AluOpType.add,
                                        op1=mybir.AluOpType.max)
            elif bias is not None:
                nc.vector.tensor_scalar(out, psum, bias, None,
                                        op0=mybir.AluOpType.add)
            else:
                nc.vector.tensor_copy(out, psum)
        else:
            self.t_act += act_ns
            if relu:
                nc.scalar.activation(out, psum, mybir.ActivationFunctionType.Relu,
                                     bias=bias)
            elif bias is not None:
                nc.scalar.activation(out, psum,
                                     mybir.ActivationFunctionType.Identity,
                                     bias=bias)
            else:
                nc.scalar.copy(out, psum)


def _build_program():
    nc = bacc.Bacc("TRN2", target_bir_lowering=False, debug=False)

    def din(name, shape, dtype):
        return nc.dram_tensor(name, list(shape), dtype, kind="ExternalInput").ap()

    xt_d = din("xt", (KT, 128, B), BF16)
    wenc_d = din("wenc", (128, 512), BF16)
    benc_d = din("benc", (128, 4), FP32)
    wd1_d = din("wd1", (128, NG * 2 * H), BF16)
    wd2_d = din("wd2", (128, UC // 2 * H), BF16)
    wd3_d = din("wd3", (128, UC // 2 * H), BF16)
    wd4_d = din("wd4", (128, UC // 2 * 32), BF16)
    bdec_d = din("bdec", (128, 96), FP32)
    out_d = nc.dram_tensor("out", [UC, B], FP32, kind="ExternalOutput").ap()

    RELU = True

    with tile.TileContext(nc) as tc:
        with (
            tc.tile_pool(name="const", bufs=1) as const,
            tc.tile_pool(name="h1p", bufs=2) as h1p,
            tc.tile_pool(name="h2p", bufs=2) as h2p,
            tc.tile_pool(name="h3p", bufs=2) as h3p,
            tc.tile_pool(name="stg", bufs=3) as stgp,
            tc.tile_pool(name="ps", bufs=4, space="PSUM") as psp,
        ):
            drain = _Drain(nc)

            def load(dst_shape, dtype, src, tag):
                t = const.tile(list(dst_shape), dtype, tag=tag, name=tag)
                nc.sync.dma_start(out=t[:], in_=src)
                return t

            # PE warm-up burst on memset data: keeps the HAM activity
            # window busy during the DMA-bound head so the encoder runs
            # at 2.4 GHz.  No DMA dependencies.
            wu = const.tile([128, 512], BF16, tag="wu", name="wu")
            nc.gpsimd.memset(wu[:], 0.0)
            wu_ps = psp.tile([128, CP], FP32, tag="ps", name="wu_ps")
            for i in range(14):
                nc.tensor.matmul(wu_ps[:, (i % 2) * CH:(i % 2) * CH + CH],
                                 wu[:, 0:128], wu[:, 0:CH])

            # encoder weights + x stream first so the encoder starts
            # while the decoder weights are still loading
            wenc = load((128, 512), BF16, wenc_d[:], "wenc")
            benc = load((128, 4), FP32, benc_d[:], "benc")
            we1 = wenc[:, 0:KT * H]
            we2 = wenc[0:H, 256:320]
            we3 = wenc[0:H, 320:384]
            we4 = wenc[0:H, 384:512]
            be1 = benc[0:H, 0:1]
            be2 = benc[0:H, 1:2]
            be3 = benc[0:H, 2:3]
            be4 = benc[:, 3:4]

            xts = [const.tile([128, B], BF16, tag=f"xt{t}", name=f"xt{t}")
                   for t in range(KT)]
            # dma_start issue costs ~0.6us of the issuing engine queue, so
            # spread issues across SP and ACT (idle at preamble) and load
            # the encoder-critical first chunks before everything else.
            qs = [nc.scalar, nc.sync, nc.scalar, nc.sync]
            for cc in range(2):
                for t in range(KT):
                    qs[t].dma_start(out=xts[t][:, cc * CH:(cc + 1) * CH],
                                    in_=xt_d[t][:, cc * CH:(cc + 1) * CH])
            wd1 = const.tile([128, NG * 2 * H], BF16, tag="wd1", name="wd1")
            nc.scalar.dma_start(out=wd1[:], in_=wd1_d[:])
            bdec = const.tile([128, 96], FP32, tag="bdec", name="bdec")
            nc.sync.dma_start(out=bdec[:], in_=bdec_d[:])
            for t in range(KT):
                qs[t].dma_start(out=xts[t][:, CP:B], in_=xt_d[t][:, CP:B])
            wd2 = const.tile([128, UC // 2 * H], BF16, tag="wd2", name="wd2")
            nc.sync.dma_start(out=wd2[:], in_=wd2_d[:])
            wd3 = const.tile([128, UC // 2 * H], BF16, tag="wd3", name="wd3")
            nc.scalar.dma_start(out=wd3[:], in_=wd3_d[:])
            wd4 = const.tile([128, UC // 2 * 32], BF16, tag="wd4", name="wd4")
            nc.sync.dma_start(out=wd4[:], in_=wd4_d[:])
            bd1 = bdec[:, 0:32]
            bd2 = bdec[:, 32:64]
            bd3 = bdec[:, 64:96]

            z1 = const.tile([H, B], BF16, tag="z1", name="z1")
            z2 = const.tile([H, B], BF16, tag="z2", name="z2")
            z3 = const.tile([H, B], BF16, tag="z3", name="z3")
            zr = const.tile([128, B], BF16, tag="zr", name="zr")

            # ---------------- encoder (replicated), 512-chunk stages -----
            # Four parallel chains (one per 512-chunk) with short drains keep
            # the encoder critical path short; reuses the decoder psum tag.
            def enc_l1c(c):
                c0 = c * CH
                ps = psp.tile([128, CP], FP32, tag="ps", name=f"pe1_{c}")
                for t in range(KT):
                    nc.tensor.matmul(ps[0:H, 0:CH], we1[:, t * H:(t + 1) * H],
                                     xts[t][:, c0:c0 + CH],
                                     start=(t == 0), stop=(t == KT - 1))
                drain(z1[:, c0:c0 + CH], ps[0:H, 0:CH], be1, RELU)

            def enc_midc(c, win, bin_, zin, zout):
                c0 = c * CH
                ps = psp.tile([128, CP], FP32, tag="ps", name=f"pem_{c}")
                nc.tensor.matmul(ps[0:H, 0:CH], win, zin[:, c0:c0 + CH])
                drain(zout[:, c0:c0 + CH], ps[0:H, 0:CH], bin_, RELU)

            def enc_l4c(c):
                c0 = c * CH
                ps = psp.tile([128, CP], FP32, tag="ps", name=f"pe4_{c}")
                nc.tensor.matmul(ps[:, 0:CH], we4, z3[:, c0:c0 + CH])
                drain(zr[:, c0:c0 + CH], ps[:, 0:CH], be4, False)

            NCHUNK = B // CH
            for st in range(NCHUNK + 3):
                for lyr in range(4):
                    c = st - lyr
                    if 0 <= c < NCHUNK:
                        if lyr == 0:
                            enc_l1c(c)
                        elif lyr == 1:
                            enc_midc(c, we2, be2, z1, z2)
                        elif lyr == 2:
                            enc_midc(c, we3, be3, z2, z3)
                        else:
                            enc_l4c(c)

            # ---------------- decoder ----------------
            # h tiles are [128, CP] per (pair-bank, chunk-pair) so that
            # inter-layer dependencies are chunk-pair-local; each phase is
            # a handful of MMs + 2 [128,1024] drains.

            def l1_phase(g, T1cp, cp, w):
                ps = [psp.tile([128, CP], FP32, tag="ps", name="pl1")
                      for _ in range(2)]
                for k in range(2):
                    q = 4 * g + 2 * w + k
                    r = q % 4
                    for cc in range(2):
                        c0 = cc * CH
                        nc.tensor.matmul(
                            ps[k][:, c0:c0 + CH],
                            wd1[32 * r:32 * r + 32, g * 128:(g + 1) * 128],
                            zr[32 * r:32 * r + 32,
                               cp * CP + c0:cp * CP + c0 + CH],
                            tile_position=(32 * r, 0))
                for k in range(2):
                    q = 4 * g + 2 * w + k
                    drain(T1cp[2 * w + k][:, :], ps[k][:, :],
                          bd1[:, q:q + 1], RELU)

            def l2_phase(g, sloc, T1cp, T2cp, cp):
                s = 2 * g + sloc
                pa = psp.tile([128, CP], FP32, tag="ps", name="pa")
                pb = psp.tile([128, CP], FP32, tag="ps", name="pb")
                pp = (pa, pb)
                for cc in range(2):
                    c0 = cc * CH
                    for j in range(4):
                        blk = 2 * s + (j >> 1)
                        nc.tensor.matmul(
                            pp[BK2[j]][64 * OH2[j]:64 * OH2[j] + 64,
                                       c0:c0 + CH],
                            wd2[64 * IH2[j]:64 * IH2[j] + 64,
                                blk * H:(blk + 1) * H],
                            T1cp[2 * sloc + TS2[j]][
                                64 * IH2[j]:64 * IH2[j] + 64, c0:c0 + CH],
                            tile_position=(64 * IH2[j], 64 * OH2[j]))
                drain(T2cp[0][:, :], pa[:, :], bd2[:, 2 * s:2 * s + 1], RELU)
                drain(T2cp[1][:, :], pb[:, :],
                      bd2[:, 2 * s + 1:2 * s + 2], RELU)

            def l3_phase(g, sloc, T2cp, T3cp, cp):
                s = 2 * g + sloc
                pa = psp.tile([128, CP], FP32, tag="ps", name="pa3")
                pb = psp.tile([128, CP], FP32, tag="ps", name="pb3")
                pp = (pa, pb)
                for cc in range(2):
                    c0 = cc * CH
                    for j in range(4):
                        blk = 2 * s + (j >> 1)
                        nc.tensor.matmul(
                            pp[BK3[j]][64 * OH3[j]:64 * OH3[j] + 64,
                                       c0:c0 + CH],
                            wd3[64 * IH3[j]:64 * IH3[j] + 64,
                                blk * H:(blk + 1) * H],
                            T2cp[TS3[j]][64 * IH3[j]:64 * IH3[j] + 64,
                                         c0:c0 + CH],
                            tile_position=(64 * IH3[j], 64 * OH3[j]))
                drain(T3cp[0][:, :], pa[:, :], bd3[:, 2 * s:2 * s + 1], RELU)
                drain(T3cp[1][:, :], pb[:, :],
                      bd3[:, 2 * s + 1:2 * s + 2], RELU)

            def l4_phase(g, T3cp, cp):
                sl = slice(cp * CP, (cp + 1) * CP)
                p4 = psp.tile([128, CP], FP32, tag="ps", name="pl4")
                for cc in range(2):
                    c0 = cc * CH
                    for cs in range(4):
                        q = 4 * g + cs
                        nc.tensor.matmul(
                            p4[32 * cs:32 * cs + 32, c0:c0 + CH],
                            wd4[:, 32 * q:32 * q + 32],
                            T3cp[cs][:, c0:c0 + CH],
                            tile_position=(0, 32 * cs))
                stg = stgp.tile([128, CP], FP32, tag="stg", name="stg")
                drain(stg[:, :], p4[:, :], None, False)
                for cs in range(4):
                    nc.sync.dma_start(
                        out=out_d[8 * g + 2 * cs:8 * g + 2 * cs + 2, sl],
                        in_=stg[32 * cs:32 * cs + 2, :])

            pend = []
            for g in range(NG):
                T1 = [[h1p.tile([128, CP], BF16, tag=f"t1_{k}_{cp}",
                                name=f"t1_{k}_{cp}") for k in range(4)]
                      for cp in range(NCP)]
                T2 = [[[h2p.tile([128, CP], BF16, tag=f"t2_{sl_}_{k}_{cp}",
                                 name=f"t2_{sl_}_{k}_{cp}") for k in range(2)]
                       for cp in range(NCP)] for sl_ in range(2)]
                T3 = [[[h3p.tile([128, CP], BF16, tag=f"t3_{sl_}_{k}_{cp}",
                                 name=f"t3_{sl_}_{k}_{cp}") for k in range(2)]
                       for cp in range(NCP)] for sl_ in range(2)]

                for cp in range(NCP):
                    for w in range(2):
                        l1_phase(g, T1[cp], cp, w)
                for sloc in range(2):
                    for cp in range(NCP):
                        l2_phase(g, sloc, T1[cp], T2[sloc][cp], cp)
                    for cp in range(NCP):
                        l3_phase(g, sloc, T2[sloc][cp], T3[sloc][cp], cp)
                pend.append((g, T3))
                if len(pend) > 1:
                    g0, T3p = pend.pop(0)
                    for cp in range(NCP):
                        l4_phase(g0, T3p[0][cp] + T3p[1][cp], cp)

            for g0, T3p in pend:
                for cp in range(NCP):
                    l4_phase(g0, T3p[0][cp] + T3p[1][cp], cp)

    nc.compile()
    return nc


def _get_program():
    global _PROG
    if _PROG is None:
        _PROG = _build_program()
    return _PROG


def kernel(x, We1, be1, We2, be2, We3, be3, We4, be4,
           Wd1, bd1, Wd2, bd2, Wd3, bd3, Wd4, bd4):
    global LAST_EXEC_NS, LAST_RESULTS
    shared = _pack_shared(np.asarray(x, np.float32),
                          np.asarray(We1, np.float32), np.asarray(be1, np.float32),
                          np.asarray(We2, np.float32), np.asarray(be2, np.float32),
                          np.asarray(We3, np.float32), np.asarray(be3, np.float32),
                          np.asarray(We4, np.float32), np.asarray(be4, np.float32))
    in_maps = []
    for c in range(NCORES):
        m = dict(shared)
        m.update(_pack_core(c, np.asarray(Wd1, np.float32), np.asarray(bd1, np.float32),
                            np.asarray(Wd2, np.float32), np.asarray(bd2, np.float32),
                            np.asarray(Wd3, np.float32), np.asarray(bd3, np.float32),
                            np.asarray(Wd4, np.float32)))
        in_maps.append(m)

    nc = _get_program()
    trace = bool(int(os.environ.get("BASSK_TRACE", "0")))
    kwargs = {}
    if trace:
        kwargs["tmpdir"] = os.environ.get("BASSK_TMPDIR") or None
    res = run_bass_kernel_spmd(nc, in_maps, core_ids=list(range(NCORES)),
                               trace=trace, **kwargs)
    LAST_EXEC_NS = res.exec_time_ns
    LAST_RESULTS = res

    outT = np.concatenate([res.results[c]["out"] for c in range(NCORES)], axis=0)
    out = outT.T.astype(np.float32) + np.asarray(bd4, np.float32)[None, :]
    return out



# revision 18
# speedup vs baseline: 1.1388x; 1.0130x over previous
"""
Trainium2 Bass kernel for nn_DisjointDecoderAE.

  encoder (shared MLP):  x[B,U] -> relu x3 -> z[B,L]
  decoder (U disjoint MLPs, stacked weights): z -> relu(64) -> relu(64) -> relu(64) -> scalar

Sharding: encoder replicated on every core (it is tiny); decoder expert-parallel
over the unit axis U (64 units per core x 8 cores).  All activations live
feature-on-partition / batch-on-free:  h^T [64, B].  Small per-unit matmuls are
packed into the 128x128 PE via tile_position quadrants (concurrent MMs).
PSUM->SBUF drain (relu+bias) is the bottleneck; drains are [128,1024] two-bank
ops (chunk-pair processing) split across VectorE and ScalarE by measured cost.

Self-contained: shapes/sharding hardcoded; host packs weights, device computes,
host re-assembles (final transpose + bd4 bias on host).
"""

import os
import sys

sys.path.insert(0, "/opt/trn_rl_repo")

import numpy as np
import ml_dtypes

import concourse.bass as bass
import concourse.mybir as mybir
import concourse.tile as tile
from concourse import bacc
from concourse.bass_utils import run_bass_kernel_spmd

B, U, L, H = 2048, 512, 32, 64
NCORES = 8
UC = U // NCORES          # 64 units per core
NG = UC // 8              # 8 groups of 8 units
CH = 512                  # one fp32 PSUM bank of batch
CP = 1024                 # chunk-pair (drain granularity, 2 banks)
NCP = B // CP             # 2 chunk-pairs
KT = U // 128             # 4 k-tiles for encoder layer 1

BF16 = mybir.dt.bfloat16
FP32 = mybir.dt.float32
NPBF = ml_dtypes.bfloat16

# L2/L3 per-subgroup placement tables (j = unit index within subgroup of 4).
# ih = input partition half, oh = output partition half, tsel = which input
# tile of the subgroup's pair, bank = which output psum tensor (0=A, 1=B).
IH2 = (0, 1, 0, 1)
OH2 = (0, 1, 1, 0)
TS2 = (0, 0, 1, 1)
BK2 = (0, 0, 1, 1)
IH3 = (0, 1, 1, 0)
OH3 = (0, 1, 0, 1)
TS3 = (0, 0, 1, 1)
BK3 = (0, 0, 1, 1)

LAST_EXEC_NS = None
LAST_RESULTS = None
_PROG = None


def _pack_shared(x, We1, be1, We2, be2, We3, be3, We4, be4):
    xT = np.ascontiguousarray(x.T).astype(NPBF)              # [U, B]
    xt = np.ascontiguousarray(xT.reshape(KT, 128, B))        # k-tiles
    wenc = np.zeros((128, 512), np.float32)
    wenc[:, 0:KT * H] = We1.reshape(KT, 128, H).transpose(1, 0, 2).reshape(
        128, KT * H)
    wenc[0:H, 256:320] = We2
    wenc[0:H, 320:384] = We3
    wenc[0:H, 384:512] = np.tile(We4, (1, 4))
    benc = np.zeros((128, 4), np.float32)
    benc[0:H, 0] = be1
    benc[0:H, 1] = be2
    benc[0:H, 2] = be3
    benc[:, 3] = np.tile(be4, 4)
    return dict(xt=xt, wenc=wenc.astype(NPBF), benc=benc)


def _pack_core(c, Wd1, bd1, Wd2, bd2, Wd3, bd3, Wd4):
    u0 = c * UC
    w1 = Wd1[u0:u0 + UC]
    b1 = bd1[u0:u0 + UC]
    w2 = Wd2[u0:u0 + UC]
    b2 = bd2[u0:u0 + UC]
    w3 = Wd3[u0:u0 + UC]
    b3 = bd3[u0:u0 + UC]
    w4 = Wd4[u0:u0 + UC]

    # L1: one [32,128] lhsT per unit-pair q (both units share rhs z):
    # row strip q%4, col block q//4 = group index.
    wd1p = np.zeros((128, NG * 2 * H), np.float32)
    bd1p = np.zeros((128, UC // 2), np.float32)
    for q in range(UC // 2):
        r = q % 4
        blk = q // 4
        wd1p[32 * r:32 * r + 32, blk * 128:blk * 128 + 64] = w1[2 * q]
        wd1p[32 * r:32 * r + 32, blk * 128 + 64:blk * 128 + 128] = w1[2 * q + 1]
        bd1p[0:64, q] = b1[2 * q]
        bd1p[64:128, q] = b1[2 * q + 1]

    NS = UC // 4  # 16 subgroups
    wd2p = np.zeros((128, NS * 2 * H), np.float32)
    wd3p = np.zeros((128, NS * 2 * H), np.float32)
    bd2p = np.zeros((128, NS * 2), np.float32)
    bd3p = np.zeros((128, NS * 2), np.float32)
    for s in range(NS):
        for j in range(4):
            u = 4 * s + j
            blk = 2 * s + (j >> 1)
            wd2p[64 * IH2[j]:64 * IH2[j] + 64, blk * H:(blk + 1) * H] = w2[u]
            wd3p[64 * IH3[j]:64 * IH3[j] + 64, blk * H:(blk + 1) * H] = w3[u]
        # T2 banks: A = {4s lo, 4s+1 hi}; B = {4s+3 lo, 4s+2 hi}
        bd2p[0:64, 2 * s] = b2[4 * s]
        bd2p[64:128, 2 * s] = b2[4 * s + 1]
        bd2p[0:64, 2 * s + 1] = b2[4 * s + 3]
        bd2p[64:128, 2 * s + 1] = b2[4 * s + 2]
        # T3 banks natural
        bd3p[0:64, 2 * s] = b3[4 * s]
        bd3p[64:128, 2 * s] = b3[4 * s + 1]
        bd3p[0:64, 2 * s + 1] = b3[4 * s + 2]
        bd3p[64:128, 2 * s + 1] = b3[4 * s + 3]

    # M padded to 32 (zero cols) so L4 matmuls cover all psum partitions
    wd4p = np.zeros((128, UC // 2 * 32), np.float32)
    for p in range(UC // 2):
        wd4p[0:64, 32 * p] = w4[2 * p]
        wd4p[64:128, 32 * p + 1] = w4[2 * p + 1]

    bdec = np.concatenate([bd1p, bd2p, bd3p], axis=1)        # [128, 96]
    return dict(wd1=wd1p.astype(NPBF), wd2=wd2p.astype(NPBF),
                wd3=wd3p.astype(NPBF), wd4=wd4p.astype(NPBF), bdec=bdec)


class _Drain:
    """Weighted VectorE/ScalarE alternation for PSUM->SBUF drains,
    using HW-measured per-op costs."""

    def __init__(self, nc):
        self.nc = nc
        self.t_dve = 0.0
        self.t_act = 0.0

    def __call__(self, out, psum, bias=None, relu=False):
        fd = 1
        for step, cnt in psum.ap[1:]:
            fd *= cnt
        dve_ns = (120.0 + fd) / 0.96 + 88.0
        act_ns = (172.0 + fd) / 1.2 + 117.0
        nc = self.nc
        if self.t_dve + dve_ns <= self.t_act + act_ns:
            self.t_dve += dve_ns
            if relu:
                nc.vector.tensor_scalar(out, psum, bias, 0.0,
                                        op0=mybir.AluOpType.add,
                                        op1=mybir.AluOpType.max)
            elif bias is not None:
                nc.vector.tensor_scalar(out, psum, bias, None,
                                        op0=mybir.AluOpType.add)
            else:
                nc.vector.tensor_copy(out, psum)
        else:
            self.t_act += act_ns
            if relu:
                nc.scalar.activation(out, psum, mybir.ActivationFunctionType.Relu,
                                     bias=bias)
            elif bias is not None:
                nc.scalar.activation(out, psum,
                                     mybir.ActivationFunctionType.Identity,
                                     bias=bias)
            else:
                nc.scalar.copy(out, psum)


def _build_program():
    nc = bacc.Bacc("TRN2", target_bir_lowering=False, debug=False)

    def din(name, shape, dtype):
        return nc.dram_tensor(name, list(shape), dtype, kind="ExternalInput").ap()

    xt_d = din("xt", (KT, 128, B), BF16)
    wenc_d = din("wenc", (128, 512), BF16)
    benc_d = din("benc", (128, 4), FP32)
    wd1_d = din("wd1", (128, NG * 2 * H), BF16)
    wd2_d = din("wd2", (128, UC // 2 * H), BF16)
    wd3_d = din("wd3", (128, UC // 2 * H), BF16)
    wd4_d = din("wd4", (128, UC // 2 * 32), BF16)
    bdec_d = din("bdec", (128, 96), FP32)
    out_d = nc.dram_tensor("out", [UC, B], FP32, kind="ExternalOutput").ap()

    RELU = True

    with tile.TileContext(nc) as tc:
        with (
            tc.tile_pool(name="const", bufs=1) as const,
            tc.tile_pool(name="h1p", bufs=2) as h1p,
            tc.tile_pool(name="h2p", bufs=2) as h2p,
            tc.tile_pool(name="h3p", bufs=2) as h3p,
            tc.tile_pool(name="stg", bufs=3) as stgp,
            tc.tile_pool(name="ps", bufs=4, space="PSUM") as psp,
        ):
            drain = _Drain(nc)

            def load(dst_shape, dtype, src, tag):
                t = const.tile(list(dst_shape), dtype, tag=tag, name=tag)
                nc.sync.dma_start(out=t[:], in_=src)
                return t

            # PE warm-up burst on memset data: keeps the HAM activity
            # window busy during the DMA-bound head so the encoder runs
            # at 2.4 GHz.  No DMA dependencies.
            wu = const.tile([128, 512], BF16, tag="wu", name="wu")
            nc.gpsimd.memset(wu[:], 0.0)
            wu_ps = psp.tile([128, CP], FP32, tag="ps", name="wu_ps")
            for i in range(14):
                nc.tensor.matmul(wu_ps[:, (i % 2) * CH:(i % 2) * CH + CH],
                                 wu[:, 0:128], wu[:, 0:CH])

            # encoder weights + x stream first so the encoder starts
            # while the decoder weights are still loading
            wenc = load((128, 512), BF16, wenc_d[:], "wenc")
            benc = load((128, 4), FP32, benc_d[:], "benc")
            we1 = wenc[:, 0:KT * H]
            we2 = wenc[0:H, 256:320]
            we3 = wenc[0:H, 320:384]
            we4 = wenc[0:H, 384:512]
            be1 = benc[0:H, 0:1]
            be2 = benc[0:H, 1:2]
            be3 = benc[0:H, 2:3]
            be4 = benc[:, 3:4]

            xts = [const.tile([128, B], BF16, tag=f"xt{t}", name=f"xt{t}")
                   for t in range(KT)]
            # dma_start issue costs ~0.6us of the issuing engine queue, so
            # spread issues across SP and ACT (idle at preamble) and load
            # the encoder-critical first chunks before everything else.
            qs = [nc.scalar, nc.sync, nc.scalar, nc.sync]
            for cc in range(2):
                for t in range(KT):
                    qs[t].dma_start(out=xts[t][:, cc * CH:(cc + 1) * CH],
                                    in_=xt_d[t][:, cc * CH:(cc + 1) * CH])
            wd1 = const.tile([128, NG * 2 * H], BF16, tag="wd1", name="wd1")
            nc.scalar.dma_start(out=wd1[:], in_=wd1_d[:])
            bdec = const.tile([128, 96], FP32, tag="bdec", name="bdec")
            nc.sync.dma_start(out=bdec[:], in_=bdec_d[:])
            for t in range(KT):
                qs[t].dma_start(out=xts[t][:, CP:B], in_=xt_d[t][:, CP:B])
            wd2 = const.tile([128, UC // 2 * H], BF16, tag="wd2", name="wd2")
            nc.sync.dma_start(out=wd2[:], in_=wd2_d[:])
            wd3 = const.tile([128, UC // 2 * H], BF16, tag="wd3", name="wd3")
            nc.scalar.dma_start(out=wd3[:], in_=wd3_d[:])
            wd4 = const.tile([128, UC // 2 * 32], BF16, tag="wd4", name="wd4")
            nc.sync.dma_start(out=wd4[:], in_=wd4_d[:])
            bd1 = bdec[:, 0:32]
            bd2 = bdec[:, 32:64]
            bd3 = bdec[:, 64:96]

            z1 = const.tile([H, B], BF16, tag="z1", name="z1")
            z2 = const.tile([H, B], BF16, tag="z2", name="z2")
            z3 = const.tile([H, B], BF16, tag="z3", name="z3")
            zr = const.tile([128, B], BF16, tag="zr", name="zr")

            # ---------------- encoder (replicated), 512-chunk stages -----
            # Four parallel chains (one per 512-chunk) with short drains keep
            # the encoder critical path short; reuses the decoder psum tag.
            def enc_l1c(c):
                c0 = c * CH
                ps = psp.tile([128, CP], FP32, tag="ps", name=f"pe1_{c}")
                for t in range(KT):
                    nc.tensor.matmul(ps[0:H, 0:CH], we1[:, t * H:(t + 1) * H],
                                     xts[t][:, c0:c0 + CH],
                                     start=(t == 0), stop=(t == KT - 1))
                drain(z1[:, c0:c0 + CH], ps[0:H, 0:CH], be1, RELU)

            def enc_midc(c, win, bin_, zin, zout):
                c0 = c * CH
                ps = psp.tile([128, CP], FP32, tag="ps", name=f"pem_{c}")
                nc.tensor.matmul(ps[0:H, 0:CH], win, zin[:, c0:c0 + CH])
                drain(zout[:, c0:c0 + CH], ps[0:H, 0:CH], bin_, RELU)

            def enc_l4c(c):
                c0 = c * CH
                ps = psp.tile([128, CP], FP32, tag="ps", name=f"pe4_{c}")
                nc.tensor.matmul(ps[:, 0:CH], we4, z3[:, c0:c0 + CH])
                drain(zr[:, c0:c0 + CH], ps[:, 0:CH], be4, False)

            NCHUNK = B // CH
            for st in range(NCHUNK + 3):
                for lyr in range(4):
                    c = st - lyr
                    if 0 <= c < NCHUNK:
                        if lyr == 0:
                            enc_l1c(c)
                        elif lyr == 1:
                            enc_midc(c, we2, be2, z1, z2)
                        elif lyr == 2:
                            enc_midc(c, we3, be3, z2, z3)
                        else:
                            enc_l4c(c)

            # ---------------- decoder ----------------
            # h tiles are [128, CP] per (pair-bank, chunk-pair) so that
            # inter-layer dependencies are chunk-pair-local; each phase is
            # a handful of MMs + 2 [128,1024] drains.

            def l1_phase(g, T1cp, cp, w):
                ps = [psp.tile([128, CP], FP32, tag="ps", name="pl1")
                      for _ in range(2)]
                for k in range(2):
                    q = 4 * g + 2 * w + k
                    r = q % 4
                    for cc in range(2):
                        c0 = cc * CH
                        nc.tensor.matmul(
                            ps[k][:, c0:c0 + CH],
                            wd1[32 * r:32 * r + 32, g * 128:(g + 1) * 128],
                            zr[32 * r:32 * r + 32,
                               cp * CP + c0:cp * CP + c0 + CH],
                            tile_position=(32 * r, 0))
                for k in range(2):
                    q = 4 * g + 2 * w + k
                    drain(T1cp[2 * w + k][:, :], ps[k][:, :],
                          bd1[:, q:q + 1], RELU)

            def l2_phase(g, sloc, T1cp, T2cp, cp):
                s = 2 * g + sloc
                pa = psp.tile([128, CP], FP32, tag="ps", name="pa")
                pb = psp.tile([128, CP], FP32, tag="ps", name="pb")
                pp = (pa, pb)
                for cc in range(2):
                    c0 = cc * CH
                    for j in range(4):
                        blk = 2 * s + (j >> 1)
                        nc.tensor.matmul(
                            pp[BK2[j]][64 * OH2[j]:64 * OH2[j] + 64,
                                       c0:c0 + CH],
                            wd2[64 * IH2[j]:64 * IH2[j] + 64,
                                blk * H:(blk + 1) * H],
                            T1cp[2 * sloc + TS2[j]][
                                64 * IH2[j]:64 * IH2[j] + 64, c0:c0 + CH],
                            tile_position=(64 * IH2[j], 64 * OH2[j]))
                drain(T2cp[0][:, :], pa[:, :], bd2[:, 2 * s:2 * s + 1], RELU)
                drain(T2cp[1][:, :], pb[:, :],
                      bd2[:, 2 * s + 1:2 * s + 2], RELU)

            def l3_phase(g, sloc, T2cp, T3cp, cp):
                s = 2 * g + sloc
                pa = psp.tile([128, CP], FP32, tag="ps", name="pa3")
                pb = psp.tile([128, CP], FP32, tag="ps", name="pb3")
                pp = (pa, pb)
                for cc in range(2):
                    c0 = cc * CH
                    for j in range(4):
                        blk = 2 * s + (j >> 1)
                        nc.tensor.matmul(
                            pp[BK3[j]][64 * OH3[j]:64 * OH3[j] + 64,
                                       c0:c0 + CH],
                            wd3[64 * IH3[j]:64 * IH3[j] + 64,
                                blk * H:(blk + 1) * H],
                            T2cp[TS3[j]][64 * IH3[j]:64 * IH3[j] + 64,
                                         c0:c0 + CH],
                            tile_position=(64 * IH3[j], 64 * OH3[j]))
                drain(T3cp[0][:, :], pa[:, :], bd3[:, 2 * s:2 * s + 1], RELU)
                drain(T3cp[1][:, :], pb[:, :],
                      bd3[:, 2 * s + 1:2 * s + 2], RELU)

            def l4_phase(g, T3cp, cp):
                sl = slice(cp * CP, (cp + 1) * CP)
                p4 = psp.tile([128, CP], FP32, tag="ps", name="pl4")
                for cc in range(2):
                    c0 = cc * CH
                    for cs in range(4):
                        q = 4 * g + cs
                        nc.tensor.matmul(
                            p4[32 * cs:32 * cs + 32, c0:c0 + CH],
                            wd4[:, 32 * q:32 * q + 32],
                            T3cp[cs][:, c0:c0 + CH],
                            tile_position=(0, 32 * cs))
                stg = stgp.tile([128, CP], FP32, tag="stg", name="stg")
                drain(stg[:, :], p4[:, :], None, False)
                for k in range(2):
                    nc.sync.dma_start(
                        out=out_d[8 * g + k:8 * g + 8:2, sl],
                        in_=stg[k:128:32, :])

            pend = []
            for g in range(NG):
                T1 = [[h1p.tile([128, CP], BF16, tag=f"t1_{k}_{cp}",
                                name=f"t1_{k}_{cp}") for k in range(4)]
                      for cp in range(NCP)]
                T2 = [[[h2p.tile([128, CP], BF16, tag=f"t2_{sl_}_{k}_{cp}",
                                 name=f"t2_{sl_}_{k}_{cp}") for k in range(2)]
                       for cp in range(NCP)] for sl_ in range(2)]
                T3 = [[[h3p.tile([128, CP], BF16, tag=f"t3_{sl_}_{k}_{cp}",
                                 name=f"t3_{sl_}_{k}_{cp}") for k in range(2)]
                       for cp in range(NCP)] for sl_ in range(2)]

                for cp in range(NCP):
                    for w in range(2):
                        l1_phase(g, T1[cp], cp, w)
                for sloc in range(2):
                    for cp in range(NCP):
                        l2_phase(g, sloc, T1[cp], T2[sloc][cp], cp)
                    for cp in range(NCP):
                        l3_phase(g, sloc, T2[sloc][cp], T3[sloc][cp], cp)
                pend.append((g, T3))
                if len(pend) > 1:
                    g0, T3p = pend.pop(0)
                    for cp in range(NCP):
                        l4_phase(g0, T3p[0][cp] + T3p[1][cp], cp)

            for g0, T3p in pend:
                for cp in range(NCP):
                    l4_phase(g0, T3p[0][cp] + T3p[1][cp], cp)

    nc.compile()
    return nc


def _get_program():
    global _PROG
    if _PROG is None:
        _PROG = _build_program()
    return _PROG


def kernel(x, We1, be1, We2, be2, We3, be3, We4, be4,
           Wd1, bd1, Wd2, bd2, Wd3, bd3, Wd4, bd4):
    global LAST_EXEC_NS, LAST_RESULTS
    shared = _pack_shared(np.asarray(x, np.float32),
                          np.asarray(We1, np.float32), np.asarray(be1, np.float32),
                          np.asarray(We2, np.float32), np.asarray(be2, np.float32),
                          np.asarray(We3, np.float32), np.asarray(be3, np.float32),
                          np.asarray(We4, np.float32), np.asarray(be4, np.float32))
    in_maps = []
    for c in range(NCORES):
        m = dict(shared)
        m.update(_pack_core(c, np.asarray(Wd1, np.float32), np.asarray(bd1, np.float32),
                            np.asarray(Wd2, np.float32), np.asarray(bd2, np.float32),
                            np.asarray(Wd3, np.float32), np.asarray(bd3, np.float32),
                            np.asarray(Wd4, np.float32)))
        in_maps.append(m)

    nc = _get_program()
    trace = bool(int(os.environ.get("BASSK_TRACE", "0")))
    kwargs = {}
    if trace:
        kwargs["tmpdir"] = os.environ.get("BASSK_TMPDIR") or None
    res = run_bass_kernel_spmd(nc, in_maps, core_ids=list(range(NCORES)),
                               trace=trace, **kwargs)
    LAST_EXEC_NS = res.exec_time_ns
    LAST_RESULTS = res

    outT = np.concatenate([res.results[c]["out"] for c in range(NCORES)], axis=0)
    out = outT.T.astype(np.float32) + np.asarray(bd4, np.float32)[None, :]
    return out



# revision 19
# speedup vs baseline: 1.1775x; 1.0340x over previous
"""
Trainium2 Bass kernel for nn_DisjointDecoderAE.

  encoder (shared MLP):  x[B,U] -> relu x3 -> z[B,L]
  decoder (U disjoint MLPs, stacked weights): z -> relu(64) -> relu(64) -> relu(64) -> scalar

Sharding: encoder replicated on every core (it is tiny); decoder expert-parallel
over the unit axis U (64 units per core x 8 cores).  All activations live
feature-on-partition / batch-on-free:  h^T [64, B].  Small per-unit matmuls are
packed into the 128x128 PE via tile_position quadrants (concurrent MMs).
PSUM->SBUF drain (relu+bias) is the bottleneck; drains are [128,1024] two-bank
ops (chunk-pair processing) split across VectorE and ScalarE by measured cost.

Self-contained: shapes/sharding hardcoded; host packs weights, device computes,
host re-assembles (final transpose + bd4 bias on host).
"""

import os
import sys

sys.path.insert(0, "/opt/trn_rl_repo")

import numpy as np
import ml_dtypes

import concourse.bass as bass
import concourse.mybir as mybir
import concourse.tile as tile
from concourse import bacc
from concourse.bass_utils import run_bass_kernel_spmd

B, U, L, H = 2048, 512, 32, 64
NCORES = 8
UC = U // NCORES          # 64 units per core
NG = UC // 8              # 8 groups of 8 units
CH = 512                  # one fp32 PSUM bank of batch
CP = 1024                 # chunk-pair (drain granularity, 2 banks)
NCP = B // CP             # 2 chunk-pairs
KT = U // 128             # 4 k-tiles for encoder layer 1

BF16 = mybir.dt.bfloat16
FP32 = mybir.dt.float32
NPBF = ml_dtypes.bfloat16

# L2/L3 per-subgroup placement tables (j = unit index within subgroup of 4).
# ih = input partition half, oh = output partition half, tsel = which input
# tile of the subgroup's pair, bank = which output psum tensor (0=A, 1=B).
IH2 = (0, 1, 0, 1)
OH2 = (0, 1, 1, 0)
TS2 = (0, 0, 1, 1)
BK2 = (0, 0, 1, 1)
IH3 = (0, 1, 1, 0)
OH3 = (0, 1, 0, 1)
TS3 = (0, 0, 1, 1)
BK3 = (0, 0, 1, 1)

LAST_EXEC_NS = None
LAST_RESULTS = None
_PROG = None


def _pack_shared(x, We1, be1, We2, be2, We3, be3, We4, be4):
    xT = np.ascontiguousarray(x.T).astype(NPBF)              # [U, B]
    xt = np.ascontiguousarray(xT.reshape(KT, 128, B))        # k-tiles
    wenc = np.zeros((128, 512), np.float32)
    wenc[:, 0:KT * H] = We1.reshape(KT, 128, H).transpose(1, 0, 2).reshape(
        128, KT * H)
    wenc[0:H, 256:320] = We2
    wenc[0:H, 320:384] = We3
    wenc[0:H, 384:512] = np.tile(We4, (1, 4))
    benc = np.zeros((128, 4), np.float32)
    benc[0:H, 0] = be1
    benc[0:H, 1] = be2
    benc[0:H, 2] = be3
    benc[:, 3] = np.tile(be4, 4)
    return dict(xt=xt, wenc=wenc.astype(NPBF), benc=benc)


def _pack_core(c, Wd1, bd1, Wd2, bd2, Wd3, bd3, Wd4):
    u0 = c * UC
    w1 = Wd1[u0:u0 + UC]
    b1 = bd1[u0:u0 + UC]
    w2 = Wd2[u0:u0 + UC]
    b2 = bd2[u0:u0 + UC]
    w3 = Wd3[u0:u0 + UC]
    b3 = bd3[u0:u0 + UC]
    w4 = Wd4[u0:u0 + UC]

    # L1: one [32,128] lhsT per unit-pair q (both units share rhs z):
    # row strip q%4, col block q//4 = group index.
    wd1p = np.zeros((128, NG * 2 * H), np.float32)
    bd1p = np.zeros((128, UC // 2), np.float32)
    for q in range(UC // 2):
        r = q % 4
        blk = q // 4
        wd1p[32 * r:32 * r + 32, blk * 128:blk * 128 + 64] = w1[2 * q]
        wd1p[32 * r:32 * r + 32, blk * 128 + 64:blk * 128 + 128] = w1[2 * q + 1]
        bd1p[0:64, q] = b1[2 * q]
        bd1p[64:128, q] = b1[2 * q + 1]

    NS = UC // 4  # 16 subgroups
    wd2p = np.zeros((128, NS * 2 * H), np.float32)
    wd3p = np.zeros((128, NS * 2 * H), np.float32)
    bd2p = np.zeros((128, NS * 2), np.float32)
    bd3p = np.zeros((128, NS * 2), np.float32)
    for s in range(NS):
        for j in range(4):
            u = 4 * s + j
            blk = 2 * s + (j >> 1)
            wd2p[64 * IH2[j]:64 * IH2[j] + 64, blk * H:(blk + 1) * H] = w2[u]
            wd3p[64 * IH3[j]:64 * IH3[j] + 64, blk * H:(blk + 1) * H] = w3[u]
        # T2 banks: A = {4s lo, 4s+1 hi}; B = {4s+3 lo, 4s+2 hi}
        bd2p[0:64, 2 * s] = b2[4 * s]
        bd2p[64:128, 2 * s] = b2[4 * s + 1]
        bd2p[0:64, 2 * s + 1] = b2[4 * s + 3]
        bd2p[64:128, 2 * s + 1] = b2[4 * s + 2]
        # T3 banks natural
        bd3p[0:64, 2 * s] = b3[4 * s]
        bd3p[64:128, 2 * s] = b3[4 * s + 1]
        bd3p[0:64, 2 * s + 1] = b3[4 * s + 2]
        bd3p[64:128, 2 * s + 1] = b3[4 * s + 3]

    # M padded to 32 (zero cols) so L4 matmuls cover all psum partitions
    wd4p = np.zeros((128, UC // 2 * 32), np.float32)
    for p in range(UC // 2):
        wd4p[0:64, 32 * p] = w4[2 * p]
        wd4p[64:128, 32 * p + 1] = w4[2 * p + 1]

    bdec = np.concatenate([bd1p, bd2p, bd3p], axis=1)        # [128, 96]
    return dict(wd1=wd1p.astype(NPBF), wd2=wd2p.astype(NPBF),
                wd3=wd3p.astype(NPBF), wd4=wd4p.astype(NPBF), bdec=bdec)


class _Drain:
    """Weighted VectorE/ScalarE alternation for PSUM->SBUF drains,
    using HW-measured per-op costs."""

    def __init__(self, nc):
        self.nc = nc
        self.t_dve = 0.0
        self.t_act = 0.0

    def __call__(self, out, psum, bias=None, relu=False):
        fd = 1
        for step, cnt in psum.ap[1:]:
            fd *= cnt
        dve_ns = (120.0 + fd) / 0.96 + 88.0
        act_ns = (172.0 + fd) / 1.2 + 117.0
        nc = self.nc
        if self.t_dve + dve_ns <= self.t_act + act_ns:
            self.t_dve += dve_ns
            if relu:
                nc.vector.tensor_scalar(out, psum, bias, 0.0,
                                        op0=mybir.AluOpType.add,
                                        op1=mybir.AluOpType.max)
            elif bias is not None:
                nc.vector.tensor_scalar(out, psum, bias, None,
                                        op0=mybir.AluOpType.add)
            else:
                nc.vector.tensor_copy(out, psum)
        else:
            self.t_act += act_ns
            if relu:
                nc.scalar.activation(out, psum, mybir.ActivationFunctionType.Relu,
                                     bias=bias)
            elif bias is not None:
                nc.scalar.activation(out, psum,
                                     mybir.ActivationFunctionType.Identity,
                                     bias=bias)
            else:
                nc.scalar.copy(out, psum)


def _build_program():
    nc = bacc.Bacc("TRN2", target_bir_lowering=False, debug=False)

    def din(name, shape, dtype):
        return nc.dram_tensor(name, list(shape), dtype, kind="ExternalInput").ap()

    xt_d = din("xt", (KT, 128, B), BF16)
    wenc_d = din("wenc", (128, 512), BF16)
    benc_d = din("benc", (128, 4), FP32)
    wd1_d = din("wd1", (128, NG * 2 * H), BF16)
    wd2_d = din("wd2", (128, UC // 2 * H), BF16)
    wd3_d = din("wd3", (128, UC // 2 * H), BF16)
    wd4_d = din("wd4", (128, UC // 2 * 32), BF16)
    bdec_d = din("bdec", (128, 96), FP32)
    out_d = nc.dram_tensor("out", [UC, B], FP32, kind="ExternalOutput").ap()

    RELU = True

    with tile.TileContext(nc) as tc:
        with (
            tc.tile_pool(name="const", bufs=1) as const,
            tc.tile_pool(name="h1p", bufs=3) as h1p,
            tc.tile_pool(name="h2p", bufs=3) as h2p,
            tc.tile_pool(name="h3p", bufs=3) as h3p,
            tc.tile_pool(name="stg", bufs=3) as stgp,
            tc.tile_pool(name="ps", bufs=4, space="PSUM") as psp,
        ):
            drain = _Drain(nc)

            def load(dst_shape, dtype, src, tag):
                t = const.tile(list(dst_shape), dtype, tag=tag, name=tag)
                nc.sync.dma_start(out=t[:], in_=src)
                return t

            # PE warm-up burst on memset data: keeps the HAM activity
            # window busy during the DMA-bound head so the encoder runs
            # at 2.4 GHz.  No DMA dependencies.
            wu = const.tile([128, 512], BF16, tag="wu", name="wu")
            nc.gpsimd.memset(wu[:], 0.0)
            wu_ps = psp.tile([128, CP], FP32, tag="ps", name="wu_ps")
            for i in range(14):
                nc.tensor.matmul(wu_ps[:, (i % 2) * CH:(i % 2) * CH + CH],
                                 wu[:, 0:128], wu[:, 0:CH])

            # encoder weights + x stream first so the encoder starts
            # while the decoder weights are still loading
            wenc = load((128, 512), BF16, wenc_d[:], "wenc")
            benc = load((128, 4), FP32, benc_d[:], "benc")
            we1 = wenc[:, 0:KT * H]
            we2 = wenc[0:H, 256:320]
            we3 = wenc[0:H, 320:384]
            we4 = wenc[0:H, 384:512]
            be1 = benc[0:H, 0:1]
            be2 = benc[0:H, 1:2]
            be3 = benc[0:H, 2:3]
            be4 = benc[:, 3:4]

            xts = [const.tile([128, B], BF16, tag=f"xt{t}", name=f"xt{t}")
                   for t in range(KT)]
            # dma_start issue costs ~0.6us of the issuing engine queue, so
            # spread issues across SP and ACT (idle at preamble) and load
            # the encoder-critical first chunks before everything else.
            qs = [nc.scalar, nc.sync, nc.scalar, nc.sync]
            for cc in range(2):
                for t in range(KT):
                    qs[t].dma_start(out=xts[t][:, cc * CH:(cc + 1) * CH],
                                    in_=xt_d[t][:, cc * CH:(cc + 1) * CH])
            wd1 = const.tile([128, NG * 2 * H], BF16, tag="wd1", name="wd1")
            nc.scalar.dma_start(out=wd1[:], in_=wd1_d[:])
            bdec = const.tile([128, 96], FP32, tag="bdec", name="bdec")
            nc.sync.dma_start(out=bdec[:], in_=bdec_d[:])
            for t in range(KT):
                qs[t].dma_start(out=xts[t][:, CP:B], in_=xt_d[t][:, CP:B])
            wd2 = const.tile([128, UC // 2 * H], BF16, tag="wd2", name="wd2")
            nc.sync.dma_start(out=wd2[:], in_=wd2_d[:])
            wd3 = const.tile([128, UC // 2 * H], BF16, tag="wd3", name="wd3")
            nc.scalar.dma_start(out=wd3[:], in_=wd3_d[:])
            wd4 = const.tile([128, UC // 2 * 32], BF16, tag="wd4", name="wd4")
            nc.sync.dma_start(out=wd4[:], in_=wd4_d[:])
            bd1 = bdec[:, 0:32]
            bd2 = bdec[:, 32:64]
            bd3 = bdec[:, 64:96]

            z1 = const.tile([H, B], BF16, tag="z1", name="z1")
            z2 = const.tile([H, B], BF16, tag="z2", name="z2")
            z3 = const.tile([H, B], BF16, tag="z3", name="z3")
            zr = const.tile([128, B], BF16, tag="zr", name="zr")

            # ---------------- encoder (replicated), 512-chunk stages -----
            # Four parallel chains (one per 512-chunk) with short drains keep
            # the encoder critical path short; reuses the decoder psum tag.
            def enc_l1c(c):
                c0 = c * CH
                ps = psp.tile([128, CP], FP32, tag="ps", name=f"pe1_{c}")
                for t in range(KT):
                    nc.tensor.matmul(ps[0:H, 0:CH], we1[:, t * H:(t + 1) * H],
                                     xts[t][:, c0:c0 + CH],
                                     start=(t == 0), stop=(t == KT - 1))
                drain(z1[:, c0:c0 + CH], ps[0:H, 0:CH], be1, RELU)

            def enc_midc(c, win, bin_, zin, zout):
                c0 = c * CH
                ps = psp.tile([128, CP], FP32, tag="ps", name=f"pem_{c}")
                nc.tensor.matmul(ps[0:H, 0:CH], win, zin[:, c0:c0 + CH])
                drain(zout[:, c0:c0 + CH], ps[0:H, 0:CH], bin_, RELU)

            def enc_l4c(c):
                c0 = c * CH
                ps = psp.tile([128, CP], FP32, tag="ps", name=f"pe4_{c}")
                nc.tensor.matmul(ps[:, 0:CH], we4, z3[:, c0:c0 + CH])
                drain(zr[:, c0:c0 + CH], ps[:, 0:CH], be4, False)

            def enc_pass(chunks):
                for st in range(len(chunks) + 3):
                    for lyr in range(4):
                        idx = st - lyr
                        if 0 <= idx < len(chunks):
                            c = chunks[idx]
                            if lyr == 0:
                                enc_l1c(c)
                            elif lyr == 1:
                                enc_midc(c, we2, be2, z1, z2)
                            elif lyr == 2:
                                enc_midc(c, we3, be3, z2, z3)
                            else:
                                enc_l4c(c)

            enc_pass([0, 1])

            # ---------------- decoder ----------------
            # h tiles are [128, CP] per (pair-bank, chunk-pair) so that
            # inter-layer dependencies are chunk-pair-local; each phase is
            # a handful of MMs + 2 [128,1024] drains.

            def l1_phase(g, T1cp, cp, w):
                ps = [psp.tile([128, CP], FP32, tag="ps", name="pl1")
                      for _ in range(2)]
                for k in range(2):
                    q = 4 * g + 2 * w + k
                    r = q % 4
                    for cc in range(2):
                        c0 = cc * CH
                        nc.tensor.matmul(
                            ps[k][:, c0:c0 + CH],
                            wd1[32 * r:32 * r + 32, g * 128:(g + 1) * 128],
                            zr[32 * r:32 * r + 32,
                               cp * CP + c0:cp * CP + c0 + CH],
                            tile_position=(32 * r, 0))
                for k in range(2):
                    q = 4 * g + 2 * w + k
                    drain(T1cp[2 * w + k][:, :], ps[k][:, :],
                          bd1[:, q:q + 1], RELU)

            def l2_phase(g, sloc, T1cp, T2cp, cp):
                s = 2 * g + sloc
                pa = psp.tile([128, CP], FP32, tag="ps", name="pa")
                pb = psp.tile([128, CP], FP32, tag="ps", name="pb")
                pp = (pa, pb)
                for cc in range(2):
                    c0 = cc * CH
                    for j in range(4):
                        blk = 2 * s + (j >> 1)
                        nc.tensor.matmul(
                            pp[BK2[j]][64 * OH2[j]:64 * OH2[j] + 64,
                                       c0:c0 + CH],
                            wd2[64 * IH2[j]:64 * IH2[j] + 64,
                                blk * H:(blk + 1) * H],
                            T1cp[2 * sloc + TS2[j]][
                                64 * IH2[j]:64 * IH2[j] + 64, c0:c0 + CH],
                            tile_position=(64 * IH2[j], 64 * OH2[j]))
                drain(T2cp[0][:, :], pa[:, :], bd2[:, 2 * s:2 * s + 1], RELU)
                drain(T2cp[1][:, :], pb[:, :],
                      bd2[:, 2 * s + 1:2 * s + 2], RELU)

            def l3_phase(g, sloc, T2cp, T3cp, cp):
                s = 2 * g + sloc
                pa = psp.tile([128, CP], FP32, tag="ps", name="pa3")
                pb = psp.tile([128, CP], FP32, tag="ps", name="pb3")
                pp = (pa, pb)
                for cc in range(2):
                    c0 = cc * CH
                    for j in range(4):
                        blk = 2 * s + (j >> 1)
                        nc.tensor.matmul(
                            pp[BK3[j]][64 * OH3[j]:64 * OH3[j] + 64,
                                       c0:c0 + CH],
                            wd3[64 * IH3[j]:64 * IH3[j] + 64,
                                blk * H:(blk + 1) * H],
                            T2cp[TS3[j]][64 * IH3[j]:64 * IH3[j] + 64,
                                         c0:c0 + CH],
                            tile_position=(64 * IH3[j], 64 * OH3[j]))
                drain(T3cp[0][:, :], pa[:, :], bd3[:, 2 * s:2 * s + 1], RELU)
                drain(T3cp[1][:, :], pb[:, :],
                      bd3[:, 2 * s + 1:2 * s + 2], RELU)

            def l4_phase(g, T3cp, cp):
                sl = slice(cp * CP, (cp + 1) * CP)
                p4 = psp.tile([128, CP], FP32, tag="ps", name="pl4")
                for cc in range(2):
                    c0 = cc * CH
                    for cs in range(4):
                        q = 4 * g + cs
                        nc.tensor.matmul(
                            p4[32 * cs:32 * cs + 32, c0:c0 + CH],
                            wd4[:, 32 * q:32 * q + 32],
                            T3cp[cs][:, c0:c0 + CH],
                            tile_position=(0, 32 * cs))
                stg = stgp.tile([128, CP], FP32, tag="stg", name="stg")
                drain(stg[:, :], p4[:, :], None, False)
                for k in range(2):
                    nc.sync.dma_start(
                        out=out_d[8 * g + k:8 * g + 8:2, sl],
                        in_=stg[k:128:32, :])

            pend = []
            for cp in range(NCP):
                for g in range(NG):
                    T1 = [h1p.tile([128, CP], BF16, tag=f"t1_{k}",
                                   name=f"t1_{g}_{cp}_{k}") for k in range(4)]
                    for w in range(2):
                        l1_phase(g, T1, cp, w)
                    if cp == 0 and g == 0:
                        # encoder chunks 2-3 overlap the first decode group
                        enc_pass([2, 3])
                    T2 = [[h2p.tile([128, CP], BF16, tag=f"t2_{sl_}_{k}",
                                    name=f"t2_{g}_{cp}_{sl_}_{k}")
                           for k in range(2)] for sl_ in range(2)]
                    T3 = [[h3p.tile([128, CP], BF16, tag=f"t3_{sl_}_{k}",
                                    name=f"t3_{g}_{cp}_{sl_}_{k}")
                           for k in range(2)] for sl_ in range(2)]
                    for sloc in range(2):
                        l2_phase(g, sloc, T1, T2[sloc], cp)
                    for sloc in range(2):
                        l3_phase(g, sloc, T2[sloc], T3[sloc], cp)
                    pend.append((g, T3[0] + T3[1], cp))
                    if len(pend) > 1:
                        l4_phase(*pend.pop(0))

            for job in pend:
                l4_phase(*job)

    nc.compile()
    return nc


def _get_program():
    global _PROG
    if _PROG is None:
        _PROG = _build_program()
    return _PROG


def kernel(x, We1, be1, We2, be2, We3, be3, We4, be4,
           Wd1, bd1, Wd2, bd2, Wd3, bd3, Wd4, bd4):
    global LAST_EXEC_NS, LAST_RESULTS
    shared = _pack_shared(np.asarray(x, np.float32),
                          np.asarray(We1, np.float32), np.asarray(be1, np.float32),
                          np.asarray(We2, np.float32), np.asarray(be2, np.float32),
                          np.asarray(We3, np.float32), np.asarray(be3, np.float32),
                          np.asarray(We4, np.float32), np.asarray(be4, np.float32))
    in_maps = []
    for c in range(NCORES):
        m = dict(shared)
        m.update(_pack_core(c, np.asarray(Wd1, np.float32), np.asarray(bd1, np.float32),
                            np.asarray(Wd2, np.float32), np.asarray(bd2, np.float32),
                            np.asarray(Wd3, np.float32), np.asarray(bd3, np.float32),
                            np.asarray(Wd4, np.float32)))
        in_maps.append(m)

    nc = _get_program()
    trace = bool(int(os.environ.get("BASSK_TRACE", "0")))
    kwargs = {}
    if trace:
        kwargs["tmpdir"] = os.environ.get("BASSK_TMPDIR") or None
    res = run_bass_kernel_spmd(nc, in_maps, core_ids=list(range(NCORES)),
                               trace=trace, **kwargs)
    LAST_EXEC_NS = res.exec_time_ns
    LAST_RESULTS = res

    outT = np.concatenate([res.results[c]["out"] for c in range(NCORES)], axis=0)
    out = outT.T.astype(np.float32) + np.asarray(bd4, np.float32)[None, :]
    return out

